# revision 15
# baseline (speedup 1.0000x reference)
"""Trainium2 Bass kernel for conv-qkv rank-1 attention (bf16 pipeline).

out = gamma * q * sum(k*v) + x, where q,k,v are per-time-slice 3x3 convs
(C=64 -> C=64) of x [B=8, C=64, T=16, W=64, H=64].

Sharding: data-parallel over B across 8 cores (1 example/core), conv
weights replicated. No cross-core communication.

Per-core schedule: T slices in pairs; slice t on SBUF partitions 0-63,
slice t+1 on 64-127 -> two concurrent PE row-group chains (K=64), which
maxes the array fill rate (1 col/cycle/chain). All matmuls are uniform
64x128 stationaries (geometry changes stall the array ~300ns). All
streams bf16, PSUM accumulates f32.

Two phases per pair so the final writeback overlaps the matmuls:
  phase 1 (k,v): chain-lo stationary [Wk|Wv], chain-hi [Wv|Wk]; evict
    with biases to bf16, small cross-DMA aligns v with k's partitions,
    fused k*v mult+pixel-sum STT accumulates s. s lands on q's
    partitions, so no swap; gamma*s is ready before phase 2 ends.
  phase 2 (q): [Wq|0] / [0|Wq]; evict, then out = q*(gamma*s) + x
    STTs + output DMAs run per 2-block chunk, overlapped with the
    remaining q matmuls. Tail after the last matmul is ~3us.

x is staged twice (interior at even and odd column offsets) so every
3x3 tap window is 4B-aligned -- unaligned bf16 moving operands cost
~20% fill rate. Host pads/converts so loads are contiguous DMAs; host
upcasts the bf16 output.
"""

import numpy as np
import ml_dtypes

import concourse.bacc as bacc
import concourse.bass as bass
import concourse.mybir as mybir
import concourse.tile as tile
from concourse import bass_utils

F32 = mybir.dt.float32
BF16 = mybir.dt.bfloat16
ALU = mybir.AluOpType
ACT = mybir.ActivationFunctionType
NPBF16 = np.dtype(ml_dtypes.bfloat16)

B, C, T, W, H = 8, 64, 16, 64, 64
WP, HP = W + 2, H + 4            # pad rows [1,65); cols [2,66) / [3,67)
NPAIR = T // 2
RB = 8                           # W-rows per pixel block
NBLK = W // RB
BN = RB * H                      # moving free dim per matmul (512)
NTAP = 9
QC = 2                           # blocks per out-writeback chunk


def _pack_weights(wq, wk, wv):
    def taps(w):  # [O, I, 1, 3, 3] -> [I, 9, O]
        return np.ascontiguousarray(
            np.asarray(w, np.float32).reshape(C, C, 9).transpose(1, 2, 0))

    wq_t, wk_t, wv_t = taps(wq), taps(wk), taps(wv)
    # phase 1: k_t -> psum 0-63, v_t -> 64-127; v_t1 -> 0-63, k_t1 -> 64-127
    wkv = np.zeros((128, NTAP, 128), np.float32)
    wkv[0:64, :, 0:64] = wk_t
    wkv[0:64, :, 64:128] = wv_t
    wkv[64:128, :, 0:64] = wv_t
    wkv[64:128, :, 64:128] = wk_t
    # phase 2 (M=64, column-placed): q_t -> psum 0-63, q_t1 -> 64-127
    wq2 = np.zeros((128, NTAP, 64), np.float32)
    wq2[0:64] = wq_t
    wq2[64:128] = wq_t
    return wkv.astype(NPBF16), wq2.astype(NPBF16)


def _emit(nc, tc, xe_d, xo_d, wkv_d, wq_d, gam_d, bkv_lo_d, bkv_hi_d, bq_d,
          out_d, ctx):
    const = ctx.enter_context(tc.tile_pool(name="const", bufs=1))
    state = ctx.enter_context(tc.tile_pool(name="state", bufs=1))
    psum = ctx.enter_context(
        tc.tile_pool(name="psum", bufs=3, space=bass.MemorySpace.PSUM))
    psumq = ctx.enter_context(
        tc.tile_pool(name="psumq", bufs=2, space=bass.MemorySpace.PSUM))
    kvpool = ctx.enter_context(tc.tile_pool(name="kvpool", bufs=4))

    wkv_t = const.tile([128, NTAP, 128], BF16, tag="wkv")
    wq_t = const.tile([128, NTAP, 64], BF16, tag="wq")
    gam_t = const.tile([128, 1], F32, tag="gam")
    bkv_lo_t = const.tile([128, 1], F32, tag="bkvlo")
    bkv_hi_t = const.tile([128, 1], F32, tag="bkvhi")
    bq_t = const.tile([128, 1], F32, tag="bq")

    nc.sync.dma_start(wkv_t[:], wkv_d[:])
    nc.sync.dma_start(wq_t[:], wq_d[:])
    nc.sync.dma_start(gam_t[:], gam_d[:])
    nc.sync.dma_start(bkv_lo_t[:], bkv_lo_d[:])
    nc.sync.dma_start(bkv_hi_t[:], bkv_hi_d[:])
    nc.sync.dma_start(bq_t[:], bq_d[:])

    xe = [state.tile([128, WP, HP], BF16, tag=f"xe{i}", name=f"xe{i}")
          for i in range(3)]
    xo = [state.tile([128, WP, HP], BF16, tag=f"xo{i}", name=f"xo{i}")
          for i in range(3)]
    qsb = [state.tile([128, NBLK, BN], BF16, tag=f"qsb{i}", name=f"qsb{i}")
           for i in range(2)]
    ot = [state.tile([128, NBLK, BN], BF16, tag=f"ot{i}", name=f"ot{i}")
          for i in range(2)]
    scr = state.tile([128, BN], BF16, tag="scr")
    sacc = [state.tile([128, NBLK], F32, tag=f"sa{i}", name=f"sa{i}")
            for i in range(2)]
    sful = [state.tile([128, 1], F32, tag=f"sf{i}", name=f"sf{i}")
            for i in range(2)]

    def load_pair(p):
        # xo first (tap 0 reads it); xe rides the gpsimd queue
        te, to = xe[p % 3], xo[p % 3]
        nc.sync.dma_start(to[0:64], xo_d[2 * p])
        nc.sync.dma_start(to[64:128], xo_d[2 * p + 1])
        nc.gpsimd.dma_start(te[0:64], xe_d[2 * p])
        nc.gpsimd.dma_start(te[64:128], xe_d[2 * p + 1])

    load_pair(0)
    if NPAIR > 1:
        load_pair(1)

    for p in range(NPAIR):
        pb = p % 2
        xe_, xo_ = xe[p % 3], xo[p % 3]
        qsb_, ot_, sacc_, sful_ = qsb[pb], ot[pb], sacc[pb], sful[pb]

        def rhs(half, tap, j):
            dy, dx = tap // 3, tap % 3
            r0 = j * RB + dy
            base = 64 * half
            if dx == 1:
                return xe_[base:base + 64, r0:r0 + RB, 2:2 + H]
            if dx == 0:
                return xo_[base:base + 64, r0:r0 + RB, 2:2 + H]
            return xo_[base:base + 64, r0:r0 + RB, 4:4 + H]

        # ---- phase 1: k,v convs + kv pixel-sum ----
        for j in range(NBLK):
            pkv_lo = psum.tile([128, BN], F32, tag="pkv_lo", name="pkv_lo")
            pkv_hi = psum.tile([128, BN], F32, tag="pkv_hi", name="pkv_hi")
            for tap in range(NTAP):
                st, sp = tap == 0, tap == NTAP - 1
                nc.tensor.matmul(pkv_lo[:, :], wkv_t[0:64, tap, :],
                                 rhs(0, tap, j), start=st, stop=sp)
                nc.tensor.matmul(pkv_hi[:, :], wkv_t[64:128, tap, :],
                                 rhs(1, tap, j), start=st, stop=sp)

            kv_lo = kvpool.tile([128, BN], BF16, tag="kv_lo", name="kv_lo")
            kv_hi = kvpool.tile([128, BN], BF16, tag="kv_hi", name="kv_hi")
            vsb = kvpool.tile([128, BN], BF16, tag="vsb", name="vsb")
            nc.scalar.activation(kv_lo[:, :], pkv_lo[:, :], ACT.Identity,
                                 bias=bkv_lo_t[:, 0:1])
            nc.scalar.activation(kv_hi[:, :], pkv_hi[:, :], ACT.Identity,
                                 bias=bkv_hi_t[:, 0:1])
            # align v with k's partitions (tiny sbuf->sbuf cross-DMA).
            # On the sync queue: sharing a queue with the out-DMAs (which
            # trail each pair's s-chain) cascades delays across pairs.
            nc.sync.dma_start(vsb[0:64, :], kv_lo[64:128, :])
            nc.sync.dma_start(vsb[64:128, :], kv_hi[0:64, :])

            nc.vector.scalar_tensor_tensor(
                out=scr[0:64, :], in0=kv_lo[0:64, :], scalar=1.0,
                in1=vsb[0:64, :], op0=ALU.mult, op1=ALU.mult,
                accum_out=sacc_[0:64, j:j + 1])
            nc.vector.scalar_tensor_tensor(
                out=scr[64:128, :], in0=kv_hi[64:128, :], scalar=1.0,
                in1=vsb[64:128, :], op0=ALU.mult, op1=ALU.mult,
                accum_out=sacc_[64:128, j:j + 1])

        nc.vector.reduce_sum(sful_[:, :], sacc_[:, :],
                             axis=mybir.AxisListType.X)
        nc.vector.tensor_scalar_mul(sful_[:, :], sful_[:, :], gam_t[:, 0:1])

        # prefetch after phase 1 so this pair's cross-DMAs go first on
        # the sync queue; the loads drain during phase 2
        if p + 2 < NPAIR:
            load_pair(p + 2)

        # ---- phase 2: q convs (M=64 col-placed, one shared tile) ----
        for j in range(NBLK):
            pq = psumq.tile([128, BN], F32, tag="pq", name="pq")
            for tap in range(NTAP):
                st, sp = tap == 0, tap == NTAP - 1
                nc.tensor.matmul(pq[0:64, :], wq_t[0:64, tap, :],
                                 rhs(0, tap, j), start=st, stop=sp)
                nc.tensor.matmul(pq[64:128, :], wq_t[64:128, tap, :],
                                 rhs(1, tap, j), start=st, stop=sp)
            nc.scalar.activation(qsb_[:, j, :], pq[:, :],
                                 ACT.Identity, bias=bq_t[:, 0:1])

            if j % QC == QC - 1:
                m = j - QC + 1
                r0 = 1 + m * RB
                # out = q * (gamma*s) + x, fused (bf16)
                nc.vector.scalar_tensor_tensor(
                    out=ot_[0:64, m:m + QC, :],
                    in0=qsb_[0:64, m:m + QC, :],
                    scalar=sful_[0:64, 0:1],
                    in1=xe_[0:64, r0:r0 + QC * RB, 2:2 + H],
                    op0=ALU.mult, op1=ALU.add)
                nc.vector.scalar_tensor_tensor(
                    out=ot_[64:128, m:m + QC, :],
                    in0=qsb_[64:128, m:m + QC, :],
                    scalar=sful_[64:128, 0:1],
                    in1=xe_[64:128, r0:r0 + QC * RB, 2:2 + H],
                    op0=ALU.mult, op1=ALU.add)
                nc.gpsimd.dma_start(
                    out_d[2 * p, :, m * BN:(m + QC) * BN],
                    ot_[0:64, m:m + QC, :])
                nc.gpsimd.dma_start(
                    out_d[2 * p + 1, :, m * BN:(m + QC) * BN],
                    ot_[64:128, m:m + QC, :])


_CACHE = {}


def _build():
    if "nc" in _CACHE:
        return _CACHE["nc"]
    nc = bacc.Bacc("TRN2", target_bir_lowering=False, debug=False,
                   enable_asserts=False, num_devices=8)
    xe_d = nc.dram_tensor("xe16", (T, C, WP, HP), BF16,
                          kind="ExternalInput").ap()
    xo_d = nc.dram_tensor("xo16", (T, C, WP, HP), BF16,
                          kind="ExternalInput").ap()
    wkv_d = nc.dram_tensor("wkv", (128, NTAP, 128), BF16,
                           kind="ExternalInput").ap()
    wq_d = nc.dram_tensor("wq2", (128, NTAP, 64), BF16,
                          kind="ExternalInput").ap()
    gam_d = nc.dram_tensor("gamma_bc", (128, 1), F32,
                           kind="ExternalInput").ap()
    bkv_lo_d = nc.dram_tensor("b_kv_lo", (128, 1), F32,
                              kind="ExternalInput").ap()
    bkv_hi_d = nc.dram_tensor("b_kv_hi", (128, 1), F32,
                              kind="ExternalInput").ap()
    bq_d = nc.dram_tensor("b_q", (128, 1), F32, kind="ExternalInput").ap()
    out_d = nc.dram_tensor("out", (T, C, W * H), BF16,
                           kind="ExternalOutput").ap()
    from contextlib import ExitStack
    with tile.TileContext(nc) as tc, ExitStack() as ctx:
        _emit(nc, tc, xe_d, xo_d, wkv_d, wq_d, gam_d, bkv_lo_d, bkv_hi_d,
              bq_d, out_d, ctx)
    nc.compile()
    _CACHE["nc"] = nc
    return nc


def run_spmd(x, wq, wk, wv, bq, bk, bv, gamma, trace=False, **kw):
    nc = _build()
    wkv, wq2 = _pack_weights(wq, wk, wv)
    bq = np.asarray(bq, np.float32).reshape(C)
    bk = np.asarray(bk, np.float32).reshape(C)
    bv = np.asarray(bv, np.float32).reshape(C)
    bkv_lo = np.concatenate([bk, bv]).reshape(128, 1)
    bkv_hi = np.concatenate([bv, bk]).reshape(128, 1)
    bqq = np.concatenate([bq, bq]).reshape(128, 1)
    gam = np.full((128, 1), np.float32(np.asarray(gamma).reshape(-1)[0]),
                  np.float32)
    x = np.asarray(x, np.float32)
    in_maps = []
    for b in range(B):
        xt = x[b].transpose(1, 0, 2, 3).astype(NPBF16)
        xe = np.zeros((T, C, WP, HP), NPBF16)
        xe[:, :, 1:1 + W, 2:2 + H] = xt
        xo = np.zeros((T, C, WP, HP), NPBF16)
        xo[:, :, 1:1 + W, 3:3 + H] = xt
        in_maps.append({"xe16": xe, "xo16": xo, "wkv": wkv, "wq2": wq2,
                        "gamma_bc": gam, "b_kv_lo": bkv_lo,
                        "b_kv_hi": bkv_hi, "b_q": bqq})
    res = bass_utils.run_bass_kernel_spmd(
        nc, in_maps, core_ids=list(range(B)), trace=trace, **kw)
    out = np.stack(
        [res.results[b]["out"].astype(np.float32)
         .reshape(T, C, W, H).transpose(1, 0, 2, 3) for b in range(B)],
        axis=0)
    return out, res


def kernel(x, wq, wk, wv, bq, bk, bv, gamma):
    out, _ = run_spmd(x, wq, wk, wv, bq, bk, bv, gamma)
    return out
